# revision 1
# baseline (speedup 1.0000x reference)
"""GCN (gather/segment-sum message passing) + mean-pool + MLP on 8 TRN2 cores.

Strategy (data-parallel over graphs, per the sharding hint):
 - nodes/graphs are sharded contiguously across 8 cores (batch is sorted);
   every edge is owned by the core owning its TARGET (col) node.
 - launch 1: each core computes y = rsqrt(deg) * (x @ W_gcn) for its node
   shard (host stages x pre-transposed so the PE contracts over in_dim).
 - host assembles the full y table (node-id order) + per-bank zero rows.
 - launch 2: per core, per source-bank (int16 gather indices limit a table
   to 32k rows -> 4 banks), edges are organized into "prefix rounds": nodes
   sorted by per-bank in-degree, round r gathers the r-th in-edge source row
   of every node that has one. Each round's dma_gather output tile is then
   POSITION-ALIGNED with the accumulator (node rank i -> partition i%128,
   column i//128), so aggregation is plain DVE adds - no scatter at all.
   Bank partials are merged by a small permute-gather through HBM scratch.
   Then z = relu(dinv*acc + b), graph mean-pool via one-hot PSUM matmuls,
   and the 64->64->2 MLP + sigmoid, all on-chip. Output (64,2) per core.
"""

import os
import sys

sys.path.insert(0, "/opt/trn_rl_repo")

import numpy as np

import concourse.bacc as bacc
import concourse.bass as bass
import concourse.mybir as mybir
import concourse.tile as tile
from concourse.bass_utils import run_bass_kernel_spmd
from concourse.vector_clock import ScopedClock

NC = 8          # cores
NB = 4          # source banks (int16 gather index limit)
CH = 1024       # gather chunk (slots per dma_gather; SWDGE ring caps ~128 descs/engine)
NQ = 2          # SWDGE queues for dma_gather round-robin
SUP = 512       # idx super-tile columns (x16 idxs)
P = 128
HID = 64
F32 = mybir.dt.float32
I16 = mybir.dt.int16

LAST_RUN_INFO = {}


def _split_multiwaits(nc, max_waits=1):
    """This walrus build rejects >1 semaphore wait per instruction; hoist
    extra waits onto same-engine NOPs placed immediately before."""
    import concourse.mybir as mb
    for f in nc.m.functions:
        for blk in f.blocks:
            insts = blk.instructions
            newlist = []
            changed = False
            for inst in insts:
                si = inst.sync_info
                waits = list(si.on_wait) if si is not None and si.on_wait else []
                if len(waits) > max_waits:
                    si.on_wait = waits[-max_waits:]
                    extra = waits[:-max_waits]
                    while extra:
                        nop = mb.InstNoOp(
                            name=f"I-mwsplit-{nc.next_id()}",
                            sync_info=mb.SyncInfo(on_wait=extra[:max_waits], on_update=[]),
                            engine=inst.engine,
                            bass_nofuse=True,
                        )
                        newlist.append(nop)
                        extra = extra[max_waits:]
                    changed = True
                newlist.append(inst)
            if changed:
                insts.clear()
                insts.extend(newlist)


_COMPILED = set()


def _run(nc, in_maps, trace=False):
    if id(nc) not in _COMPILED:
        nc.compile()
        _split_multiwaits(nc)
        _COMPILED.add(id(nc))
    kw = {}
    if trace:
        kw = dict(trace=True)
    try:
        return run_bass_kernel_spmd(nc, in_maps, list(range(NC)), **kw)
    except Exception:
        # transient device-unrecoverable (wedged core from an earlier run)
        import time as _time
        _time.sleep(10)
        return run_bass_kernel_spmd(nc, in_maps, list(range(NC)), **kw)


def _pjrt_runner(nc, in_maps):
    """Build the shard_map-jitted bass_exec callable ONCE with device-resident
    inputs; returns run_once() whose wall time is dispatch + device exec only
    (fresh donated zero-outputs are re-supplied per call; for benchmarking)."""
    import jax
    import numpy as _np
    from concourse import bass2jax as b2j

    b2j.install_neuronx_cc_hook()
    partition_name = nc.partition_id_tensor.name if nc.partition_id_tensor else None
    in_names, out_names, out_avals, zero_outs = [], [], [], []
    for alloc in nc.m.functions[0].allocations:
        if not isinstance(alloc, mybir.MemoryLocationSet):
            continue
        name = alloc.memorylocations[0].name
        if alloc.kind == "ExternalInput":
            if name != partition_name:
                in_names.append(name)
        elif alloc.kind == "ExternalOutput":
            shape = tuple(alloc.tensor_shape)
            dtype = mybir.dt.np(alloc.dtype)
            out_names.append(name)
            out_avals.append(jax.core.ShapedArray(shape, dtype))
            zero_outs.append(_np.zeros(shape, dtype))
    n_params, n_outs = len(in_names), len(out_avals)
    all_in = list(in_names) + out_names + ([partition_name] if partition_name else [])

    def _body(*args):
        operands = list(args)
        if partition_name is not None:
            operands.append(b2j.partition_id_tensor())
        outs = b2j._bass_exec_p.bind(
            *operands, out_avals=tuple(out_avals), in_names=tuple(all_in),
            out_names=tuple(out_names), lowering_input_output_aliases=(),
            sim_require_finite=True, sim_require_nnan=True, nc=nc)
        return tuple(outs)

    devices = jax.devices()[:NC]
    mesh = b2j.Mesh(_np.asarray(devices), ("core",))
    donate = tuple(range(n_params, n_params + n_outs))
    sharded = jax.jit(
        b2j.shard_map(_body, mesh=mesh,
                      in_specs=(b2j.PartitionSpec("core"),) * (n_params + n_outs),
                      out_specs=(b2j.PartitionSpec("core"),) * n_outs,
                      check_rep=False),
        donate_argnums=donate, keep_unused=True)
    concat_in = [
        jax.device_put(
            _np.concatenate([_np.asarray(m[name]) for m in in_maps], axis=0))
        for name in in_names
    ]
    for a in concat_in:
        a.block_until_ready()

    def run_once():
        zs = [_np.zeros((NC * z.shape[0], *z.shape[1:]), z.dtype) for z in zero_outs]
        outs = sharded(*concat_in, *zs)
        for o in outs:
            o.block_until_ready()
        return outs

    return run_once


# ---------------------------------------------------------------- launch 1


def _build_launch1(C):
    """y_tile = dinv * (x @ W);  x staged transposed [128(in), C*128(node)]."""
    nc = bacc.Bacc("TRN2", target_bir_lowering=False, debug=False)
    xT = nc.declare_dram_parameter("xT", [P, C * P], F32, isOutput=False)
    degn = nc.declare_dram_parameter("degn", [P, C], F32, isOutput=False)
    w = nc.declare_dram_parameter("w", [P, HID], F32, isOutput=False)
    ysb = nc.declare_dram_parameter("ysb", [P, C * HID], F32, isOutput=True)

    reps = int(os.environ.get("GCN_REPS", "1"))
    with tile.TileContext(nc) as tc:
        with (
            tc.tile_pool(name="sb", bufs=1) as sb,
            tc.tile_pool(name="sbx", bufs=3) as sbx,
            tc.tile_pool(name="ps", bufs=4, space="PSUM") as psp,
        ):
          for _rep in range(reps):
            w_t = sb.tile([P, HID], F32)
            nc.scalar.dma_start(out=w_t[:], in_=w[:, :])
            deg_t = sb.tile([P, C], F32)
            nc.scalar.dma_start(out=deg_t[:], in_=degn[:, :])
            dinv = sb.tile([P, C], F32)
            nc.scalar.activation(dinv[:], deg_t[:], mybir.ActivationFunctionType.Sqrt)
            nc.vector.reciprocal(dinv[:], dinv[:])
            y_t = sb.tile([P, C, HID], F32)
            for t in range(C):
                xt_t = sbx.tile([P, P], F32)
                nc.scalar.dma_start(out=xt_t[:], in_=xT[:, t * P:(t + 1) * P])
                ps = psp.tile([P, HID], F32, space="PSUM")
                nc.tensor.matmul(out=ps[:], lhsT=xt_t[:], rhs=w_t[:],
                                 start=True, stop=True)
                nc.vector.tensor_tensor(
                    out=y_t[:, t, :], in0=ps[:],
                    in1=dinv[:, t:t + 1].broadcast_to([P, HID]),
                    op=mybir.AluOpType.mult)
            nc.scalar.dma_start(out=ysb[:, :], in_=y_t[:].rearrange("p c h -> p (c h)"))
    return nc


# ---------------------------------------------------------------- launch 2


def _build_launch2(C, VB, bank_chunks, merge_chunks, n_w16):
    """bank_chunks: per bank, list of (idx_off16, nidx, [(gcol, zcol, ncols)..])
    merge_chunks: per bank, list of (idx_off16, nidx, gcol0, zcol0)
    n_w16: total idx columns (int16 words / 16)."""
    nc = bacc.Bacc("TRN2", target_bir_lowering=False, debug=False,
                   num_swdge_queues=NQ)
    ytab = nc.declare_dram_parameter("ytab", [NB * VB, HID], F32, isOutput=False)
    idxs = nc.declare_dram_parameter("idxs", [P, n_w16], I16, isOutput=False)
    degz = nc.declare_dram_parameter("degz", [P, C], F32, isOutput=False)
    gl = nc.declare_dram_parameter("gl", [P, C], F32, isOutput=False)
    iota = nc.declare_dram_parameter("iota", [P, HID], F32, isOutput=False)
    brep = nc.declare_dram_parameter("brep", [P, HID], F32, isOutput=False)
    w1a = nc.declare_dram_parameter("w1a", [P, HID], F32, isOutput=False)
    w2a = nc.declare_dram_parameter("w2a", [P, 2], F32, isOutput=False)
    iden = nc.declare_dram_parameter("iden", [P, P], F32, isOutput=False)
    out = nc.declare_dram_parameter("out", [HID, 2], F32, isOutput=True)
    dbg = os.environ.get("GCN_DEBUG") == "1"
    if dbg:
        zdbg = nc.declare_dram_parameter("zdbg", [P, C * HID], F32, isOutput=True)
    zscr = nc.dram_tensor("zscr", [NB * P * C, HID], F32)

    reps = int(os.environ.get("GCN_REPS", "1"))
    with tile.TileContext(nc) as tc:
        with (
            tc.tile_pool(name="sb", bufs=1) as sb,
            tc.tile_pool(name="stage", bufs=int(os.environ.get("GCN_SBUFS", "3"))) as stage,
            tc.tile_pool(name="idxp", bufs=3) as idxp,
            tc.tile_pool(name="ohp", bufs=3) as ohp,
            tc.tile_pool(name="ps", bufs=1, space="PSUM") as psp,
            tc.tile_pool(name="ps2", bufs=1, space="PSUM") as psp2,
        ):
            acc = sb.tile([P, C, HID], F32, tag="acc")
            z = sb.tile([P, C, HID], F32, tag="z")
            sup_state = {"s0": -1, "tile": None}

            def get_idx(off16, w):
                if sup_state["s0"] < 0 or off16 + w > sup_state["s0"] + SUP:
                    w2 = min(SUP, n_w16 - off16)
                    t = idxp.tile([P, SUP], I16, tag="idx")
                    nc.scalar.dma_start(out=t[:, :w2], in_=idxs[:, off16:off16 + w2])
                    sup_state["s0"] = off16
                    sup_state["tile"] = t
                o = off16 - sup_state["s0"]
                return sup_state["tile"][:, o:o + w]

            gq = [0]

            def gather(dst_ap, src_ap, off16, nidx):
                it = get_idx(off16, nidx // 16)
                gi = nc.gpsimd.dma_gather(dst_ap, src_ap, it, nidx, nidx, HID,
                                          queue_num=gq[0] % NQ)
                gq[0] += 1
                return gi

            def body():
              # (indented 2: repeated GCN_REPS times for benchmarking)
              sup_state["s0"] = -1
              dump_insts = []
              for b in range(NB):
                  nc.gpsimd.memset(acc[:], 0.0)
                  for (off16, nidx, pieces) in bank_chunks[b]:
                      st = stage.tile([P, CH // P, HID], F32, tag="st")
                      gather(st[:, : nidx // P, :], ytab[b * VB:(b + 1) * VB, :],
                             off16, nidx)
                      for (gcol, zcol, ncols) in pieces:
                          nc.vector.tensor_tensor(
                              out=acc[:, zcol:zcol + ncols, :],
                              in0=acc[:, zcol:zcol + ncols, :],
                              in1=st[:, gcol:gcol + ncols, :],
                              op=mybir.AluOpType.add)
                  di = nc.scalar.dma_start(
                      out=zscr[b * P * C:(b + 1) * P * C, :],
                      in_=acc[:].rearrange("p c h -> p (c h)"))
                  dump_insts.append(di)
              # merge partials (node order): z = sum_b permute(acc_b)
              nc.gpsimd.memset(z[:], 0.0)
              for b in range(NB):
                  for (off16, nidx, gcol0, zcol0) in merge_chunks[b]:
                      st = stage.tile([P, CH // P, HID], F32, tag="st")
                      gi = gather(st[:, : nidx // P, :],
                                  zscr[b * P * C:(b + 1) * P * C, :], off16, nidx)
                      tile.add_dep_helper(gi.ins, dump_insts[b].ins, sync=True,
                                          reason="merge gather reads zscr dump")
                      nc.vector.tensor_tensor(
                          out=z[:, zcol0:zcol0 + nidx // P, :],
                          in0=z[:, zcol0:zcol0 + nidx // P, :],
                          in1=st[:, : nidx // P, :],
                          op=mybir.AluOpType.add)
              # dinv
              deg_t = sb.tile([P, C], F32)
              nc.scalar.dma_start(out=deg_t[:], in_=degz[:, :])
              dinv = sb.tile([P, C], F32)
              nc.scalar.activation(dinv[:], deg_t[:], mybir.ActivationFunctionType.Sqrt)
              nc.vector.reciprocal(dinv[:], dinv[:])
              brep_t = sb.tile([P, HID], F32)
              nc.scalar.dma_start(out=brep_t[:], in_=brep[:, :])
              for c in range(C):
                  nc.vector.tensor_tensor(
                      out=z[:, c, :], in0=z[:, c, :],
                      in1=dinv[:, c:c + 1].broadcast_to([P, HID]),
                      op=mybir.AluOpType.mult)
                  nc.vector.tensor_tensor(
                      out=z[:, c, :], in0=z[:, c, :], in1=brep_t[:],
                      op=mybir.AluOpType.add)
              zf = z[:].rearrange("p c h -> p (c h)")
              nc.scalar.activation(zf, zf, mybir.ActivationFunctionType.Relu)
              if dbg:
                  nc.scalar.dma_start(out=zdbg[:, :], in_=zf)
              # pooling: one-hot PSUM matmuls
              gl_t = sb.tile([P, C], F32)
              nc.scalar.dma_start(out=gl_t[:], in_=gl[:, :])
              iota_t = sb.tile([P, HID], F32)
              nc.scalar.dma_start(out=iota_t[:], in_=iota[:, :])
              ones_t = sb.tile([P, 1], F32)
              nc.gpsimd.memset(ones_t[:], 1.0)
              ps_sum = psp.tile([HID, HID], F32, space="PSUM", tag="pssum")
              ps_cnt = psp.tile([HID, 1], F32, space="PSUM", tag="pscnt")
              for c in range(C):
                  oh = ohp.tile([P, HID], F32, tag="oh")
                  nc.vector.tensor_tensor(
                      out=oh[:], in0=gl_t[:, c:c + 1].broadcast_to([P, HID]),
                      in1=iota_t[:], op=mybir.AluOpType.is_equal)
                  nc.tensor.matmul(out=ps_sum[:], lhsT=oh[:], rhs=z[:, c, :],
                                   start=(c == 0), stop=(c == C - 1),
                                   skip_group_check=True)
                  nc.tensor.matmul(out=ps_cnt[:], lhsT=oh[:], rhs=ones_t[:],
                                   start=(c == 0), stop=(c == C - 1),
                                   skip_group_check=True)
              cnt = sb.tile([HID, 1], F32)
              nc.vector.tensor_scalar_max(cnt[:], ps_cnt[:], 1.0)
              nc.vector.reciprocal(cnt[:], cnt[:])
              g_sb = sb.tile([HID, HID], F32)
              nc.vector.tensor_tensor(out=g_sb[:], in0=ps_sum[:],
                                      in1=cnt[:].broadcast_to([HID, HID]),
                                      op=mybir.AluOpType.mult)
              # MLP with homogeneous-coordinate bias
              iden_t = sb.tile([P, P], F32)
              nc.scalar.dma_start(out=iden_t[:], in_=iden[:, :])
              w1_t = sb.tile([P, HID], F32)
              nc.scalar.dma_start(out=w1_t[:], in_=w1a[:, :])
              w2_t = sb.tile([P, 2], F32)
              nc.scalar.dma_start(out=w2_t[:], in_=w2a[:, :])
              gT = psp2.tile([HID, HID], F32, space="PSUM", tag="tr")
              nc.tensor.transpose(out=gT[:], in_=g_sb[:], identity=iden_t[:HID, :HID])
              a1 = sb.tile([P, HID], F32)
              nc.gpsimd.memset(a1[HID:HID + 1, :], 1.0)
              nc.vector.tensor_copy(a1[:HID, :], gT[:])
              h_ps = psp2.tile([HID, HID], F32, space="PSUM", tag="mm")
              nc.tensor.matmul(out=h_ps[:], lhsT=a1[0:HID + 1, :], rhs=w1_t[0:HID + 1, :],
                               start=True, stop=True)
              h_sb = sb.tile([HID, HID], F32)
              nc.scalar.activation(h_sb[:], h_ps[:], mybir.ActivationFunctionType.Relu)
              hT = psp2.tile([HID, HID], F32, space="PSUM", tag="tr2")
              nc.tensor.transpose(out=hT[:], in_=h_sb[:], identity=iden_t[:HID, :HID])
              a2 = sb.tile([P, HID], F32)
              nc.gpsimd.memset(a2[HID:HID + 1, :], 1.0)
              nc.vector.tensor_copy(a2[:HID, :], hT[:])
              o_ps = psp2.tile([HID, 2], F32, space="PSUM", tag="mm2")
              nc.tensor.matmul(out=o_ps[:], lhsT=a2[0:HID + 1, :], rhs=w2_t[0:HID + 1, :],
                               start=True, stop=True)
              o_sb = sb.tile([HID, 2], F32)
              nc.scalar.activation(o_sb[:], o_ps[:], mybir.ActivationFunctionType.Sigmoid)
              nc.scalar.dma_start(out=out[:, :], in_=o_sb[:])

            for _rep in range(reps):
                body()
    return nc


# ---------------------------------------------------------------- host glue


def _wrap16(vals):
    """int16 stream -> [128, ceil(n/16)] ucode layout (16-wrapped, 8x repl)."""
    n = len(vals)
    w = (n + 15) // 16
    a = np.full(w * 16, -1, np.int16)
    a[:n] = vals
    blk = a.reshape(w, 16).T
    return np.tile(blk, (8, 1))


def kernel(x, edge_index, batch, W_gcn, b_gcn, W1, b1, W2, b2):
    x = np.ascontiguousarray(np.asarray(x, dtype=np.float32))
    ei = np.asarray(edge_index).astype(np.int64)
    batch_np = np.asarray(batch).astype(np.int64)
    W_gcn = np.asarray(W_gcn, np.float32); b_gcn = np.asarray(b_gcn, np.float32)
    W1 = np.asarray(W1, np.float32); b1 = np.asarray(b1, np.float32)
    W2 = np.asarray(W2, np.float32); b2 = np.asarray(b2, np.float32)

    N = x.shape[0]
    G = 512
    BS = (N + NB - 1) // NB          # nodes per source bank
    VB = BS + 1                      # +1 zero row per bank
    row = ei[0].astype(np.int64)
    col = ei[1].astype(np.int64)
    # self loops appended
    sl = np.arange(N, dtype=np.int64)
    row2 = np.concatenate([row, sl])
    col2 = np.concatenate([col, sl])
    deg = np.bincount(col2, minlength=N).astype(np.float32)  # >=1 always

    gpc = G // NC
    gb = np.searchsorted(batch_np, np.arange(0, G + 1, gpc))
    Ncs = np.diff(gb)
    C = int((Ncs.max() + P - 1) // P)

    # ---------------- launch 1: y shards
    in1 = []
    for c in range(NC):
        lo, hi = int(gb[c]), int(gb[c + 1])
        n = hi - lo
        xT = np.zeros((P, C * P), np.float32)
        xT[:, :n] = x[lo:hi].T
        dg = np.ones((P, C), np.float32)
        dgf = dg.reshape(-1, order="F")      # (p,t) -> t*128+p
        dgf[:n] = deg[lo:hi]
        dg = dgf.reshape(C, P).T.copy()
        in1.append({"xT": xT, "degn": dg, "w": W_gcn})
    nc1 = _build_launch1(C)
    trace = os.environ.get("GCN_TRACE") == "1"
    r1 = _run(nc1, in1, trace=trace)
    LAST_RUN_INFO["exec1_ns"] = r1.exec_time_ns
    y_full = np.empty((N, HID), np.float32)
    for c in range(NC):
        lo, hi = int(gb[c]), int(gb[c + 1])
        ys = r1.results[c]["ysb"].reshape(P, C, HID).transpose(1, 0, 2).reshape(-1, HID)
        y_full[lo:hi] = ys[: hi - lo]
    ytab = np.zeros((NB * VB, HID), np.float32)
    for b in range(NB):
        nlo, nhi = b * BS, min((b + 1) * BS, N)
        ytab[b * VB: b * VB + (nhi - nlo)] = y_full[nlo:nhi]

    # ---------------- per-core schedules (common across cores)
    core_data = []
    for c in range(NC):
        lo, hi = int(gb[c]), int(gb[c + 1])
        m = (col2 >= lo) & (col2 < hi)
        r_c = row2[m]
        cl = (col2[m] - lo).astype(np.int64)
        bank = np.minimum(r_c // BS, NB - 1)
        core_data.append((lo, hi, r_c, cl, bank))

    # common round schedule per bank: N_br = max over cores of roundup128(n_br)
    nbr_all = []          # [NB][core] -> array of n_br
    for b in range(NB):
        per_core = []
        for c in range(NC):
            lo, hi, r_c, cl, bank = core_data[c]
            nloc = hi - lo
            degb = np.bincount(cl[bank == b], minlength=nloc)
            if degb.max() == 0:
                per_core.append(np.zeros(0, np.int64))
                continue
            h = np.bincount(degb)            # h[d] = #nodes with degb == d
            nbr = nloc - np.cumsum(h)[:-1] if len(h) > 1 else np.zeros(0, np.int64)
            # n_br = #{deg_b > r} for r = 0..max-1
            nbr = (nloc - np.cumsum(h))[:len(h) - 1]
            per_core.append(np.asarray(nbr, np.int64))
        nbr_all.append(per_core)
    bank_rounds = []      # [NB] -> padded common N_br (cols of 128)
    for b in range(NB):
        R = max((len(a) for a in nbr_all[b]), default=0)
        Nbr = np.zeros(R, np.int64)
        for a in nbr_all[b]:
            aa = np.zeros(R, np.int64)
            aa[:len(a)] = a
            Nbr = np.maximum(Nbr, ((aa + P - 1) // P) * P)
        bank_rounds.append(Nbr)

    # chunk schedule (common): per bank, chunks of <=CH slots + round pieces
    bank_chunks = []
    bank_off16 = []       # idx tensor column offset for each bank stream
    off16 = 0
    for b in range(NB):
        Nbr = bank_rounds[b]
        S = int(Nbr.sum())
        starts = np.concatenate([[0], np.cumsum(Nbr)])
        chunks = []
        pos = 0
        while pos < S:
            ln = min(CH, S - pos)
            pieces = []
            for r in range(len(Nbr)):
                a = max(pos, starts[r]); e = min(pos + ln, starts[r + 1])
                if a < e:
                    pieces.append((int((a - pos) // P), int((a - starts[r]) // P),
                                   int((e - a) // P)))
            chunks.append((off16 + pos // 16, int(ln), pieces))
            pos += ln
        bank_chunks.append(chunks)
        bank_off16.append(off16)
        off16 += S // 16
    # merge chunks (common): C*128 idxs per bank
    merge_chunks = []
    merge_off16 = []
    for b in range(NB):
        Sm = C * P
        chunks = []
        pos = 0
        while pos < Sm:
            ln = min(CH, Sm - pos)
            chunks.append((off16 + pos // 16, int(ln), int(pos // P), int(pos // P)))
            pos += ln
        merge_chunks.append(chunks)
        merge_off16.append(off16)
        off16 += Sm // 16
    n_w16 = off16

    # ---------------- per-core idx streams
    in2 = []
    iota64 = np.tile(np.arange(HID, dtype=np.float32), (P, 1))
    brep = np.tile(b_gcn[None, :], (P, 1)).astype(np.float32)
    w1a = np.zeros((P, HID), np.float32); w1a[:HID] = W1; w1a[HID] = b1
    w2a = np.zeros((P, 2), np.float32); w2a[:HID] = W2; w2a[HID] = b2
    iden = np.eye(P, dtype=np.float32)
    for c in range(NC):
        lo, hi, r_c, cl, bank = core_data[c]
        nloc = hi - lo
        idxbuf = np.empty(n_w16 * 16, np.int16)
        for b in range(NB):
            Nbr = bank_rounds[b]
            S = int(Nbr.sum())
            starts = np.concatenate([[0], np.cumsum(Nbr)])
            stream = np.full(S, BS, np.int16)          # dummy -> zero row
            mb = bank == b
            rb, clb = r_c[mb], cl[mb]
            degb = np.bincount(clb, minlength=nloc)
            order = np.argsort(-degb, kind="stable")   # bank-rank -> node
            rank = np.empty(nloc, np.int64)
            rank[order] = np.arange(nloc)
            rk = rank[clb]
            o = np.lexsort((np.arange(len(rk)), rk))
            rk_s, src_s = rk[o], (rb[o] - b * BS)
            grp_start = np.searchsorted(rk_s, rk_s)    # first occurrence index
            j = np.arange(len(rk_s)) - grp_start
            stream[starts[j] + rk_s] = src_s.astype(np.int16)
            idxbuf[bank_off16[b] * 16: bank_off16[b] * 16 + S] = stream
            # merge idx for this bank: node order -> acc_b row
            jb = rank                                   # node i -> bank rank
            mrow = (jb % P) * C + (jb // P)
            mstream = np.zeros(C * P, np.int16)
            mstream[:nloc] = mrow.astype(np.int16)
            idxbuf[merge_off16[b] * 16: merge_off16[b] * 16 + C * P] = mstream
        idxw = _wrap16(idxbuf)                          # [32, n_w16]
        dgz = np.ones(C * P, np.float32); dgz[:nloc] = deg[lo:hi]
        glv = np.full(C * P, float(HID), np.float32)
        glv[:nloc] = (batch_np[lo:hi] - c * gpc).astype(np.float32)
        in2.append({
            "ytab": ytab, "idxs": idxw,
            "degz": dgz.reshape(C, P).T.copy(),
            "gl": glv.reshape(C, P).T.copy(),
            "iota": iota64, "brep": brep, "w1a": w1a, "w2a": w2a, "iden": iden,
        })

    LAST_RUN_INFO["launch2_args"] = (C, VB, bank_chunks, merge_chunks, n_w16)
    LAST_RUN_INFO["in2"] = in2
    LAST_RUN_INFO["in1"] = in1
    LAST_RUN_INFO["C"] = C
    nc2 = _build_launch2(C, VB, bank_chunks, merge_chunks, n_w16)
    r2 = _run(nc2, in2, trace=trace)
    LAST_RUN_INFO["exec2_ns"] = r2.exec_time_ns
    if os.environ.get("GCN_DEBUG") == "1":
        LAST_RUN_INFO["zdbg"] = [r2.results[c]["zdbg"].reshape(P, C, HID) for c in range(NC)]
        LAST_RUN_INFO["gb"] = gb
        LAST_RUN_INFO["C"] = C
    out = np.concatenate([r2.results[c]["out"] for c in range(NC)], axis=0)
    return out[:G].astype(np.float32)

